# revision 24
# baseline (speedup 1.0000x reference)
"""Trainium2 Bass kernel for nn_DiffusionLoss (B=4, N=2048).

Decomposition
-------------
loss = align_term + bond_term, pooled over the batch, then scaled by the
per-sample ht factor.

* align term + all O(N) reductions -> host numpy in f64 (tiny).
* bond term: sum_ij w_i w_j (dp_ij - dg_ij)^2 expands to analytic O(N)
  sums plus the cross term P = sum_ij w_i w_j dp_ij dg_ij, which needs the
  full N x N pairwise pass -> device.

Device trick: with augmented 26-vectors
  V_i = [w_i^2 * (vp_i (x) vg_i), sqrt(c)],  U_j = [w_j^2 * (up_j (x) ug_j), sqrt(c)]
one fp32r matmul produces psum_ij = (w_i w_j)^2 d2p_ij d2g_ij + c per
entry, so a single per-entry sqrt gives the fully weighted summand and a
per-partition row-sum accumulator can merge rows from ANY row-blocks.
The +c bias (c = 4) keeps entries positive against rounding noise before
sqrt; its systematic effect (~1e-3 relative) is far inside the 2e-2 gate.

Triangle at 128-row granularity: row-block bi covers columns
[bi*128, 2048).  Its own 128-wide diagonal block is matmul'd against a
quarter-scaled copy of U (sqrt -> half weight), which makes the host
factor a uniform 2x for every device entry; the within-block sums plus
the quarter/bias corrections are reconstructed exactly on the host in
f64 from a cheap O(N*128) pass.  Row-blocks whose column count is an odd
multiple of 128 are padded with one 128-wide dummy column block (zeros
with a sqrt(c) pad row -> each entry contributes exactly sqrt(c), an
exact host-side subtraction), so every matmul piece is 256 or 512 wide:
no fp32r narrow-matmul penalty and all pieces sit inside one PSUM bank.

Pipeline: PE fills one 2048-col half of PSUM (a "generation" = one or
two whole row-block slots) while THREE engines concurrently consume the
previous half, each doing a fused sqrt + row-sum in one instruction:
  ACT  activation(Sqrt, accum_out)            0.833 ns/col
  DVE  tensor_scalar(pow 0.5, op1=add, accum) 1.042 ns/col
  Pool tensor_scalar(pow 0.5, op1=add, accum) 0.833 ns/col (no bubble)
The 2048 cols of each generation are split 512/640/896 (ACT/DVE/Pool) to
equalize finish times.  The ACT Sqrt table is pre-loaded during the DMA
window so no reader pays the 1.4us table-load.

Sharding: one program for all 8 cores; core c -> batch c//2, parity c%2
picks 8 of the 16 row-blocks.  Both parities' padded widths are the same
multiset [16,14,12,10,8,6,4,2] x 128, so a single instruction stream
serves all cores; only the host-built uv payload differs.  Slots are
paired (widest+narrowest) into four 26-row groups stacked on partitions
0/32/64/96 of one [128, 2560] fp32r SBUF tensor, which keeps the DMA
cost at 2560 B/partition/queue across the three DGE queues (SP/ACT/Pool)
and satisfies matmul's equal-base-partition constraint for lhsT and rhs.

Raw Bass (no TileContext): all waits are standalone wait_ge instructions
and each compute instruction carries at most one sem update.  The final
result DMAs carry a sem nobody waits on: the engines halt as soon as the
descriptors are issued and the runtime drains the queues.
"""

from contextlib import ExitStack

import numpy as np

import concourse.bass as bass
from concourse import mybir
from concourse.bass_utils import run_bass_kernel_spmd

B = 4
N = 2048
SIGMA_DATA = 16.0
C_BIAS = 4.0
SQC = 2.0  # sqrt(C_BIAS)

F32 = mybir.dt.float32
F32R = mybir.dt.float32r

# Row-block slots per parity, ascending bi == descending width.
SLOTS_BY_PARITY = {
    0: [0, 3, 4, 7, 8, 11, 12, 15],
    1: [1, 2, 5, 6, 9, 10, 13, 14],
}
# Padded widths (units of 128 cols) per slot index — identical for both
# parities: u = 16-bi rounded up to even.
PW = [16, 14, 12, 10, 8, 6, 4, 2]
# Three 26-partition groups at base partitions 0/32/64 (matmul requires
# lhsT/rhs base partition in {0, 32, 64}).  Each group: its slots' V
# blocks first, then their rhs spans (small slots' rhs early so the DMA
# stream in ascending column order matches PE consumption order).
GROUPS = {0: [0, 5, 7], 1: [1, 3], 2: [2, 4, 6]}
SLOT_GROUP = {}   # slot -> (group, v_col)
RHS_BASE = {}     # slot -> rhs col base within group
UV_COLS = 3456
for _g, _ss in GROUPS.items():
    for _i, _s in enumerate(_ss):
        SLOT_GROUP[_s] = (_g, 128 * _i)
_rhs_order = {0: [0, 7, 5], 1: [1, 3], 2: [4, 2, 6]}
for _g, _ss in _rhs_order.items():
    _off = 128 * len(GROUPS[_g])
    for _s in _ss:
        RHS_BASE[_s] = _off
        _off += PW[_s] * 128

# Generations: (psum_base, [slot indices]).  gen0 is the smallest slot so
# the ACT reader chain starts as early as possible.
GENS = [(0, [4]), (2048, [0]), (0, [1, 7]), (2048, [2, 6]), (0, [3]),
        (2048, [5])]
GEN_COLS = [1024, 2048, 2048, 2048, 1280, 768]
# Real TRN2: ACT Sqrt(negative) is NaN and fp32r noise reaches ~1e2 on
# large-magnitude rows, so every entry is clamped first.  DVE relu-copies
# PSUM into SBUF (the only engine that can both read PSUM and clamp);
# ACT then does the fused sqrt + row-sum from SBUF.
READER_SPLIT = [(c,) for c in GEN_COLS]

# DMA stripes: (queue, col_lo, col_hi) in issue order per queue.
STRIPES = {
    "sp": [(0, 128), (384, 640), (1152, 1536), (2048, 2432), (2944, 3456)],
    "act": [(128, 384), (1536, 2048)],
    "pool": [(640, 1152), (2432, 2944)],
}


def _piece_plan():
    """Matmul pieces: (slot, sbuf_off_in_rhs, psum_abs_off, width), in PE
    issue order.  gen0's slot is split 256/256/512/512/512 for an earlier
    start; everything else splits at PSUM bank boundaries into 256/512s."""
    pieces = []
    for gi, (base, slots) in enumerate(GENS):
        off_in_gen = 0
        for s in slots:
            cols = PW[s] * 128
            pos = 0
            while pos < cols:
                abs_off = base + off_in_gen + pos
                room = 512 - (abs_off % 512)
                w = min(512, cols - pos, room)
                if gi == 0:
                    w = min(w, 256)  # 256-wide pieces so PE starts early
                pieces.append((s, pos, abs_off, w))
                pos += w
            off_in_gen += cols
    return pieces


PIECES = _piece_plan()
# cumulative piece count at end of each generation
M_CUM = []
_n = 0
for _base, _slots in GENS:
    for _s in _slots:
        _cols = PW[_s] * 128
        _pos = 0
        while _pos < _cols:
            _pc_abs = PIECES[_n][2]
            _n += 1
            _pos += PIECES[_n - 1][3]
    M_CUM.append(_n)
assert M_CUM[-1] == len(PIECES), M_CUM


def _stripe_for(col):
    """(queue, index>=1) of the stripe containing col."""
    for q, lst in STRIPES.items():
        for i, (lo, hi) in enumerate(lst):
            if lo <= col < hi:
                return q, i + 1
    raise AssertionError(col)


def _piece_stripe_targets():
    """Per piece, the set of (queue, stripe_index) stripes it needs.  DMAs
    on one queue can complete out of order, so each stripe has its own
    semaphore and pieces wait on exactly the stripes they touch."""
    targets = []
    for s, pos, abs_off, w in PIECES:
        need = set()
        g, vcol = SLOT_GROUP[s]
        need.add(_stripe_for(vcol))
        lo = RHS_BASE[s] + pos
        col = lo
        while col < lo + w:
            q, i = _stripe_for(col)
            need.add((q, i))
            col = STRIPES[q][i - 1][1]
        targets.append(sorted(need))
    return targets


PIECE_TARGETS = _piece_stripe_targets()


def _reader_pe_targets():
    """Per generation, for each reader range, the cumulative PE piece count
    needed (pieces are in PE issue order)."""
    out = []
    for gi, (base, slots) in enumerate(GENS):
        lo_pc = M_CUM[gi - 1] if gi else 0
        gen_pieces = [(k + 1, PIECES[k][2], PIECES[k][2] + PIECES[k][3])
                      for k in range(lo_pc, M_CUM[gi])]
        ranges = [(base, base + GEN_COLS[gi])]
        tgt = []
        for lo, hi in ranges:
            need = max(idx for idx, plo, phi in gen_pieces if plo < hi and phi > lo)
            tgt.append(need)
        out.append((ranges, tgt))
    return out


READER_PLAN = _reader_pe_targets()

_NC_CACHE = None


def _build_nc():
    nc = bass.Bass("TRN2", target_bir_lowering=False, debug=False, num_devices=8)

    uv = nc.declare_dram_parameter("uv", [128, UV_COLS], F32R, isOutput=False)
    res = nc.declare_dram_parameter("res", [128, 6], F32, isOutput=True)

    clamp_off = [0, 1024, 3072, 5120, 7168, 8448]
    with (
        nc.sbuf_tensor([128, UV_COLS], F32R) as uv_t,
        nc.sbuf_tensor([128, 9216], F32) as scl,
        nc.sbuf_tensor([128, 16], F32) as res_t,
        nc.sbuf_tensor([128, 1], F32) as bias_t,
        nc.psum_tensor([128, 4096], F32) as ps,
        ExitStack() as stack,
        nc.Block() as block,
    ):
        names = ["pe_s", "act_s", "dve_s", "bias_s", "dout"]
        for q, lst in STRIPES.items():
            names += [f"q_{q}_{i + 1}" for i in range(len(lst))]
        sems = {name: stack.enter_context(nc.semaphore(name)) for name in names}
        pe_s, act_s, dve_s, bias_s, dout = (sems[k] for k in
                                            ("pe_s", "act_s", "dve_s",
                                             "bias_s", "dout"))
        qsem = {(q, i + 1): sems[f"q_{q}_{i + 1}"]
                for q, lst in STRIPES.items() for i in range(len(lst))}

        def rhs_ap(s, pos, w):
            g, _ = SLOT_GROUP[s]
            lo = RHS_BASE[s] + pos
            return uv_t[32 * g: 32 * g + 26, lo: lo + w]

        def lhs_ap(s):
            g, vcol = SLOT_GROUP[s]
            return uv_t[32 * g: 32 * g + 26, vcol: vcol + 128]

        @block.sync
        def _(sync):
            for i, (lo, hi) in enumerate(STRIPES["sp"]):
                sync.dma_start(out=uv_t[:, lo:hi], in_=uv[:, lo:hi]).then_inc(
                    qsem[("sp", i + 1)], 16)

        @block.tensor
        def _(tensor):
            waited = set()
            for gi in range(len(GENS)):
                lo_pc = M_CUM[gi - 1] if gi else 0
                for k in range(lo_pc, M_CUM[gi]):
                    s, pos, abs_off, w = PIECES[k]
                    if k == lo_pc and gi >= 2:
                        # psum half reuse: the clamp is the psum reader
                        tensor.wait_ge(dve_s, gi - 1)
                    for st in PIECE_TARGETS[k]:
                        if st not in waited:
                            tensor.wait_ge(qsem[st], 16)
                            waited.add(st)
                    nc.tensor.matmul(
                        ps[:, abs_off: abs_off + w],
                        lhs_ap(s),
                        rhs_ap(s, pos, w),
                        start=True,
                        stop=True,
                    ).then_inc(pe_s, 1)

        @block.gpsimd
        def _(gp):
            gp.memset(bias_t[:, :], 0.0).then_inc(bias_s, 1)
            for i, (lo, hi) in enumerate(STRIPES["pool"]):
                gp.dma_start(out=uv_t[:, lo:hi], in_=uv[:, lo:hi]).then_inc(
                    qsem[("pool", i + 1)], 16)

        @block.vector
        def _(vector):
            for gi, (ranges, tgt) in enumerate(READER_PLAN):
                lo, hi = ranges[0]
                vector.wait_ge(pe_s, tgt[0])
                nc.vector.tensor_scalar_max(
                    scl[:, clamp_off[gi]: clamp_off[gi] + (hi - lo)],
                    ps[:, lo:hi],
                    0.0,
                ).then_inc(dve_s, 1)

        @block.scalar
        def _(scalar):
            for i, (lo, hi) in enumerate(STRIPES["act"]):
                scalar.dma_start(out=uv_t[:, lo:hi], in_=uv[:, lo:hi]).then_inc(
                    qsem[("act", i + 1)], 16)
            # pre-load the Sqrt activation table during the DMA window
            scalar.wait_ge(bias_s, 1)
            nc.scalar.activation(
                out=res_t[:, 15:16], in_=bias_t[:, 0:1],
                func=mybir.ActivationFunctionType.Sqrt,
                bias=bias_t[:, 0:1],
            )
            for gi, (ranges, tgt) in enumerate(READER_PLAN):
                lo, hi = ranges[0]
                scalar.wait_ge(dve_s, gi + 1)
                nc.scalar.activation(
                    out=scl[:, clamp_off[gi]: clamp_off[gi] + (hi - lo)],
                    in_=scl[:, clamp_off[gi]: clamp_off[gi] + (hi - lo)],
                    func=mybir.ActivationFunctionType.Sqrt,
                    bias=bias_t[:, 0:1],
                    accum_out=res_t[:, gi: gi + 1],
                ).then_inc(act_s, 1)
            scalar.wait_ge(act_s, len(GENS))
            scalar.dma_start(out=res[:, 0:len(GENS)],
                             in_=res_t[:, 0:len(GENS)]).then_inc(dout, 16)

    return nc


def _augmented(xp32, xg32, w32):
    """U26 [B,N,26] (j side) and V26 [B,N,26] (i side), both w^2-scaled
    with a sqrt(c) pad so psum = (w_i w_j)^2 d2p d2g + c."""
    xp = xp32.astype(np.float64)
    xg = xg32.astype(np.float64)
    w = w32.astype(np.float64)
    sp = (xp * xp).sum(-1)
    sg = (xg * xg).sum(-1)
    ones = np.ones((B, N, 1))
    up = np.concatenate([xp, sp[..., None], ones], -1)
    ug = np.concatenate([xg, sg[..., None], ones], -1)
    vp = np.concatenate([-2.0 * xp, ones, sp[..., None]], -1)
    vg = np.concatenate([-2.0 * xg, ones, sg[..., None]], -1)
    U = np.einsum("bna,bnc->bnac", up, ug).reshape(B, N, 25) * (w ** 2)[..., None]
    V = np.einsum("bna,bnc->bnac", vp, vg).reshape(B, N, 25) * (w ** 2)[..., None]
    U26 = np.concatenate([U, np.full((B, N, 1), SQC)], -1).astype(np.float32)
    V26 = np.concatenate([V, np.full((B, N, 1), SQC)], -1).astype(np.float32)
    return U26, V26


def _host_inputs(U26, V26):
    in_maps = []
    for core in range(8):
        b, h = core // 2, core % 2
        slots = SLOTS_BY_PARITY[h]
        buf = np.zeros((128, UV_COLS), np.float32)
        for s in range(8):
            bi = slots[s]
            u = 16 - bi
            g, vcol = SLOT_GROUP[s]
            r0 = 32 * g
            buf[r0:r0 + 26, vcol:vcol + 128] = V26[b, bi * 128:(bi + 1) * 128].T
            # rhs: quarter-scaled diag block | U tail | optional dummy
            lo = RHS_BASE[s]
            buf[r0:r0 + 26, lo:lo + 128] = 0.25 * U26[b, bi * 128:(bi + 1) * 128].T
            tail = U26[b, (bi + 1) * 128:].T  # [26, (15-bi)*128]
            buf[r0:r0 + 26, lo + 128:lo + u * 128] = tail
            if u % 2 == 1:
                dummy = np.zeros((26, 128), np.float32)
                dummy[25, :] = SQC
                buf[r0:r0 + 26, lo + u * 128:lo + (u + 1) * 128] = dummy
        in_maps.append({"uv": np.ascontiguousarray(buf)})
    return in_maps


def _host_corrections(xp, xg, w):
    """Per-batch within-block corrections, f64.
    Returns (Wfull_dev, Wtrue):
      Wfull_dev[b] = sum over 16 diag blocks of 0.5*sqrt((w_i w_j)^2 d2p d2g + c)
                     over ALL ordered (i, j) incl. i==j  (device content)
      Wtrue[b]     = sum over blocks of w_i w_j dp dg over i != j ordered."""
    X = xp.reshape(B, 16, 128, 3)
    G = xg.reshape(B, 16, 128, 3)
    W = w.reshape(B, 16, 128)
    d2p = ((X[:, :, :, None, :] - X[:, :, None, :, :]) ** 2).sum(-1)
    d2g = ((G[:, :, :, None, :] - G[:, :, None, :, :]) ** 2).sum(-1)
    wp = (W[:, :, :, None] * W[:, :, None, :]) ** 2
    prod = wp * d2p * d2g
    wfull = 0.5 * np.sqrt(prod + C_BIAS).sum(axis=(1, 2, 3))
    m = np.sqrt(prod)
    idx = np.arange(128)
    m[:, :, idx, idx] = 0.0
    wtrue = m.sum(axis=(1, 2, 3))
    return wfull, wtrue


def _host_assemble(xp32, xg32, ht32, w32, P):
    """Alignment loss + analytic bond parts + final scaling (f64)."""
    xp = xp32.astype(np.float64)
    xg = xg32.astype(np.float64)
    ht = ht32.astype(np.float64)
    w = w32.astype(np.float64)

    W = w.sum(axis=1)
    mu = (w[..., None] * xg).sum(axis=1) / W[:, None]
    muGT = (w[..., None] * xp).sum(axis=1) / W[:, None]
    xc = xg - mu[:, None, :]
    xGTc = xp - muGT[:, None, :]
    M = np.einsum("bni,bnj->bij", w[..., None] * xGTc, xc)
    U, _, Vh = np.linalg.svd(M)
    R = U @ Vh
    det = np.linalg.det(R)
    Fm = np.diag([1.0, 1.0, -1.0])
    Rfix = np.einsum("bij,jk,bkl->bil", U, Fm, Vh)
    R = np.where(det[:, None, None] < 0, Rfix, R)
    xalign = np.einsum("bnj,bkj->bnk", xc, R) + muGT[:, None, :]
    lnum = (np.linalg.norm(xp - xalign, axis=-1) * w).sum()
    loss_align = lnum / W.sum()

    sp = (xp * xp).sum(-1)
    sg = (xg * xg).sum(-1)
    wxp = np.einsum("bn,bni->bi", w, xp)
    wxg = np.einsum("bn,bni->bi", w, xg)
    Ap = 2 * (W * (w * sp).sum(1) - (wxp ** 2).sum(1))
    Bg = 2 * (W * (w * sg).sum(1) - (wxg ** 2).sum(1))

    bond = (Ap + Bg - 2 * P).sum() / (W ** 2).sum()
    loss = loss_align + bond
    out = (ht ** 2 + SIGMA_DATA ** 2) / (ht + SIGMA_DATA) ** 2 * loss
    return out.astype(np.float32)


def kernel(xpred_l, xGT_l, ht, w_l):
    global _NC_CACHE
    xp32 = np.ascontiguousarray(np.asarray(xpred_l, dtype=np.float32))
    xg32 = np.ascontiguousarray(np.asarray(xGT_l, dtype=np.float32))
    ht32 = np.asarray(ht, dtype=np.float32)
    w32 = np.ascontiguousarray(np.asarray(w_l, dtype=np.float32))

    if _NC_CACHE is None:
        _NC_CACHE = _build_nc()
    nc = _NC_CACHE

    U26, V26 = _augmented(xp32, xg32, w32)
    in_maps = _host_inputs(U26, V26)
    results = run_bass_kernel_spmd(nc, in_maps, list(range(8))).results

    # Device: res[p, k] = per-partition accumulators (15 per core); every
    # entry already carries its w_i w_j weight, so S_dev = plain sum.
    S_dev = np.zeros(B)
    for core in range(8):
        S_dev[core // 2] += results[core]["res"].astype(np.float64).sum()

    # Dummy columns: 4 per core, each 128x128 entries of exactly sqrt(c).
    dummy_sub = 2 * 4 * 128 * 128 * SQC  # per batch (2 cores)

    xp64 = xp32.astype(np.float64)
    xg64 = xg32.astype(np.float64)
    w64 = w32.astype(np.float64)
    wfull, wtrue = _host_corrections(xp64, xg64, w64)
    P = 2.0 * (S_dev - dummy_sub - wfull) + wtrue

    return _host_assemble(xp32, xg32, ht32, w32, P)


# revision 25
# speedup vs baseline: 1.0108x; 1.0108x over previous
"""Trainium2 Bass kernel for nn_DiffusionLoss (B=4, N=2048).

Decomposition
-------------
loss = align_term + bond_term, pooled over the batch, then scaled by the
per-sample ht factor.

* align term + all O(N) reductions -> host numpy in f64 (tiny).
* bond term: sum_ij w_i w_j (dp_ij - dg_ij)^2 expands to analytic O(N)
  sums plus the cross term P = sum_ij w_i w_j dp_ij dg_ij, which needs the
  full N x N pairwise pass -> device.

Device trick: with augmented 26-vectors
  V_i = [w_i^2 * (vp_i (x) vg_i), sqrt(c)],  U_j = [w_j^2 * (up_j (x) ug_j), sqrt(c)]
one fp32r matmul produces psum_ij = (w_i w_j)^2 d2p_ij d2g_ij + c per
entry, so a single per-entry sqrt gives the fully weighted summand and a
per-partition row-sum accumulator can merge rows from ANY row-blocks.
The +c bias (c = 4) keeps entries positive against rounding noise before
sqrt; its systematic effect (~1e-3 relative) is far inside the 2e-2 gate.

Triangle at 128-row granularity: row-block bi covers columns
[bi*128, 2048).  Its own 128-wide diagonal block is matmul'd against a
quarter-scaled copy of U (sqrt -> half weight), which makes the host
factor a uniform 2x for every device entry; the within-block sums plus
the quarter/bias corrections are reconstructed exactly on the host in
f64 from a cheap O(N*128) pass.  Row-blocks whose column count is an odd
multiple of 128 are padded with one 128-wide dummy column block (zeros
with a sqrt(c) pad row -> each entry contributes exactly sqrt(c), an
exact host-side subtraction), so every matmul piece is 256 or 512 wide:
no fp32r narrow-matmul penalty and all pieces sit inside one PSUM bank.

Pipeline: PE fills one 2048-col half of PSUM (a "generation" = one or
two whole row-block slots) while THREE engines concurrently consume the
previous half, each doing a fused sqrt + row-sum in one instruction:
  ACT  activation(Sqrt, accum_out)            0.833 ns/col
  DVE  tensor_scalar(pow 0.5, op1=add, accum) 1.042 ns/col
  Pool tensor_scalar(pow 0.5, op1=add, accum) 0.833 ns/col (no bubble)
The 2048 cols of each generation are split 512/640/896 (ACT/DVE/Pool) to
equalize finish times.  The ACT Sqrt table is pre-loaded during the DMA
window so no reader pays the 1.4us table-load.

Sharding: one program for all 8 cores; core c -> batch c//2, parity c%2
picks 8 of the 16 row-blocks.  Both parities' padded widths are the same
multiset [16,14,12,10,8,6,4,2] x 128, so a single instruction stream
serves all cores; only the host-built uv payload differs.  Slots are
paired (widest+narrowest) into four 26-row groups stacked on partitions
0/32/64/96 of one [128, 2560] fp32r SBUF tensor, which keeps the DMA
cost at 2560 B/partition/queue across the three DGE queues (SP/ACT/Pool)
and satisfies matmul's equal-base-partition constraint for lhsT and rhs.

Raw Bass (no TileContext): all waits are standalone wait_ge instructions
and each compute instruction carries at most one sem update.  The final
result DMAs carry a sem nobody waits on: the engines halt as soon as the
descriptors are issued and the runtime drains the queues.
"""

from contextlib import ExitStack

import numpy as np

import concourse.bass as bass
from concourse import mybir
from concourse.bass_utils import run_bass_kernel_spmd

B = 4
N = 2048
SIGMA_DATA = 16.0
C_BIAS = 4.0
SQC = 2.0  # sqrt(C_BIAS)

F32 = mybir.dt.float32
F32R = mybir.dt.float32r

# Row-block slots per parity, ascending bi == descending width.
SLOTS_BY_PARITY = {
    0: [0, 3, 4, 7, 8, 11, 12, 15],
    1: [1, 2, 5, 6, 9, 10, 13, 14],
}
# Padded widths (units of 128 cols) per slot index — identical for both
# parities: u = 16-bi rounded up to even.
PW = [16, 14, 12, 10, 8, 6, 4, 2]
# Three 26-partition groups at base partitions 0/32/64 (matmul requires
# lhsT/rhs base partition in {0, 32, 64}).  Each group: its slots' V
# blocks first, then their rhs spans (small slots' rhs early so the DMA
# stream in ascending column order matches PE consumption order).
GROUPS = {0: [0, 5, 7], 1: [1, 3], 2: [2, 4, 6]}
SLOT_GROUP = {}   # slot -> (group, v_col)
RHS_BASE = {}     # slot -> rhs col base within group
UV_COLS = 3456
for _g, _ss in GROUPS.items():
    for _i, _s in enumerate(_ss):
        SLOT_GROUP[_s] = (_g, 128 * _i)
_rhs_order = {0: [0, 7, 5], 1: [1, 3], 2: [4, 2, 6]}
for _g, _ss in _rhs_order.items():
    _off = 128 * len(GROUPS[_g])
    for _s in _ss:
        RHS_BASE[_s] = _off
        _off += PW[_s] * 128

# Generations: (psum_base, [slot indices]).  gen0 is the smallest slot so
# the ACT reader chain starts as early as possible.
GENS = [(0, [4]), (2048, [0]), (0, [1, 7]), (2048, [2, 6]), (0, [3]),
        (2048, [5])]
GEN_COLS = [1024, 2048, 2048, 2048, 1280, 768]
# Real TRN2: ACT Sqrt(negative) is NaN and fp32r noise reaches ~1e2 on
# large-magnitude rows, so every entry is clamped first.  DVE relu-copies
# PSUM into SBUF (the only engine that can both read PSUM and clamp);
# ACT then does the fused sqrt + row-sum from SBUF.
READER_SPLIT = [(c,) for c in GEN_COLS]

# DMA stripes: (queue, col_lo, col_hi) in issue order per queue.
STRIPES = {
    "sp": [(0, 128), (384, 640), (1152, 1536), (2048, 2432), (2944, 3456)],
    "act": [(128, 384), (1536, 2048)],
    "pool": [(640, 1152), (2432, 2944)],
}


def _piece_plan():
    """Matmul pieces: (slot, sbuf_off_in_rhs, psum_abs_off, width), in PE
    issue order.  gen0's slot is split 256/256/512/512/512 for an earlier
    start; everything else splits at PSUM bank boundaries into 256/512s."""
    pieces = []
    for gi, (base, slots) in enumerate(GENS):
        off_in_gen = 0
        for s in slots:
            cols = PW[s] * 128
            pos = 0
            while pos < cols:
                abs_off = base + off_in_gen + pos
                room = 512 - (abs_off % 512)
                w = min(512, cols - pos, room)
                if gi == 0:
                    w = min(w, 256)  # 256-wide pieces so PE starts early
                pieces.append((s, pos, abs_off, w))
                pos += w
            off_in_gen += cols
    return pieces


PIECES = _piece_plan()
# cumulative piece count at end of each generation
M_CUM = []
_n = 0
for _base, _slots in GENS:
    for _s in _slots:
        _cols = PW[_s] * 128
        _pos = 0
        while _pos < _cols:
            _pc_abs = PIECES[_n][2]
            _n += 1
            _pos += PIECES[_n - 1][3]
    M_CUM.append(_n)
assert M_CUM[-1] == len(PIECES), M_CUM


def _stripe_for(col):
    """(queue, index>=1) of the stripe containing col."""
    for q, lst in STRIPES.items():
        for i, (lo, hi) in enumerate(lst):
            if lo <= col < hi:
                return q, i + 1
    raise AssertionError(col)


def _piece_stripe_targets():
    """Per piece, the set of (queue, stripe_index) stripes it needs.  DMAs
    on one queue can complete out of order, so each stripe has its own
    semaphore and pieces wait on exactly the stripes they touch."""
    targets = []
    for s, pos, abs_off, w in PIECES:
        need = set()
        g, vcol = SLOT_GROUP[s]
        need.add(_stripe_for(vcol))
        lo = RHS_BASE[s] + pos
        col = lo
        while col < lo + w:
            q, i = _stripe_for(col)
            need.add((q, i))
            col = STRIPES[q][i - 1][1]
        targets.append(sorted(need))
    return targets


PIECE_TARGETS = _piece_stripe_targets()


def _reader_units():
    """Flat list of reader units (psum_lo, psum_hi, pe_target, clamp_off,
    gen).  gen0 is split in two so the first clamp starts after only two
    matmul pieces, pulling the whole saturated DVE chain earlier."""
    units = []
    off = 0
    for gi, (base, slots) in enumerate(GENS):
        lo_pc = M_CUM[gi - 1] if gi else 0
        gen_pieces = [(k + 1, PIECES[k][2], PIECES[k][2] + PIECES[k][3])
                      for k in range(lo_pc, M_CUM[gi])]
        if gi == 0:
            ranges = [(base, base + GEN_COLS[0] // 2),
                      (base + GEN_COLS[0] // 2, base + GEN_COLS[0])]
        else:
            ranges = [(base, base + GEN_COLS[gi])]
        for lo, hi in ranges:
            need = max(idx for idx, plo, phi in gen_pieces if plo < hi and phi > lo)
            units.append((lo, hi, need, off, gi))
            off += hi - lo
    return units


READER_UNITS = _reader_units()
# number of DVE units completed at the end of each generation (for PE's
# psum-half-reuse waits)
DVE_CUM = []
for _gi in range(len(GENS)):
    DVE_CUM.append(sum(1 for u in READER_UNITS if u[4] <= _gi))

_NC_CACHE = None


def _build_nc():
    nc = bass.Bass("TRN2", target_bir_lowering=False, debug=False, num_devices=8)

    uv = nc.declare_dram_parameter("uv", [128, UV_COLS], F32R, isOutput=False)
    res = nc.declare_dram_parameter("res", [128, 7], F32, isOutput=True)

    with (
        nc.sbuf_tensor([128, UV_COLS], F32R) as uv_t,
        nc.sbuf_tensor([128, 9216], F32) as scl,
        nc.sbuf_tensor([128, 16], F32) as res_t,
        nc.sbuf_tensor([128, 1], F32) as bias_t,
        nc.psum_tensor([128, 4096], F32) as ps,
        ExitStack() as stack,
        nc.Block() as block,
    ):
        names = ["pe_s", "act_s", "dve_s", "bias_s", "dout"]
        for q, lst in STRIPES.items():
            names += [f"q_{q}_{i + 1}" for i in range(len(lst))]
        sems = {name: stack.enter_context(nc.semaphore(name)) for name in names}
        pe_s, act_s, dve_s, bias_s, dout = (sems[k] for k in
                                            ("pe_s", "act_s", "dve_s",
                                             "bias_s", "dout"))
        qsem = {(q, i + 1): sems[f"q_{q}_{i + 1}"]
                for q, lst in STRIPES.items() for i in range(len(lst))}

        def rhs_ap(s, pos, w):
            g, _ = SLOT_GROUP[s]
            lo = RHS_BASE[s] + pos
            return uv_t[32 * g: 32 * g + 26, lo: lo + w]

        def lhs_ap(s):
            g, vcol = SLOT_GROUP[s]
            return uv_t[32 * g: 32 * g + 26, vcol: vcol + 128]

        @block.sync
        def _(sync):
            for i, (lo, hi) in enumerate(STRIPES["sp"]):
                sync.dma_start(out=uv_t[:, lo:hi], in_=uv[:, lo:hi]).then_inc(
                    qsem[("sp", i + 1)], 16)

        @block.tensor
        def _(tensor):
            waited = set()
            for gi in range(len(GENS)):
                lo_pc = M_CUM[gi - 1] if gi else 0
                for k in range(lo_pc, M_CUM[gi]):
                    s, pos, abs_off, w = PIECES[k]
                    if k == lo_pc and gi >= 2:
                        # psum half reuse: the clamp is the psum reader
                        tensor.wait_ge(dve_s, DVE_CUM[gi - 2])
                    for st in PIECE_TARGETS[k]:
                        if st not in waited:
                            tensor.wait_ge(qsem[st], 16)
                            waited.add(st)
                    nc.tensor.matmul(
                        ps[:, abs_off: abs_off + w],
                        lhs_ap(s),
                        rhs_ap(s, pos, w),
                        start=True,
                        stop=True,
                    ).then_inc(pe_s, 1)

        @block.gpsimd
        def _(gp):
            gp.memset(bias_t[:, :], 0.0).then_inc(bias_s, 1)
            for i, (lo, hi) in enumerate(STRIPES["pool"]):
                gp.dma_start(out=uv_t[:, lo:hi], in_=uv[:, lo:hi]).then_inc(
                    qsem[("pool", i + 1)], 16)

        @block.vector
        def _(vector):
            for lo, hi, tgt, coff, gi in READER_UNITS:
                vector.wait_ge(pe_s, tgt)
                nc.vector.tensor_scalar_max(
                    scl[:, coff: coff + (hi - lo)],
                    ps[:, lo:hi],
                    0.0,
                ).then_inc(dve_s, 1)

        @block.scalar
        def _(scalar):
            for i, (lo, hi) in enumerate(STRIPES["act"]):
                scalar.dma_start(out=uv_t[:, lo:hi], in_=uv[:, lo:hi]).then_inc(
                    qsem[("act", i + 1)], 16)
            # pre-load the Sqrt activation table during the DMA window
            scalar.wait_ge(bias_s, 1)
            nc.scalar.activation(
                out=res_t[:, 15:16], in_=bias_t[:, 0:1],
                func=mybir.ActivationFunctionType.Sqrt,
                bias=bias_t[:, 0:1],
            )
            for ui, (lo, hi, tgt, coff, gi) in enumerate(READER_UNITS):
                scalar.wait_ge(dve_s, ui + 1)
                nc.scalar.activation(
                    out=scl[:, coff: coff + (hi - lo)],
                    in_=scl[:, coff: coff + (hi - lo)],
                    func=mybir.ActivationFunctionType.Sqrt,
                    bias=bias_t[:, 0:1],
                    accum_out=res_t[:, ui: ui + 1],
                ).then_inc(act_s, 1)
            scalar.wait_ge(act_s, len(READER_UNITS))
            scalar.dma_start(out=res[:, 0:len(READER_UNITS)],
                             in_=res_t[:, 0:len(READER_UNITS)]).then_inc(dout, 16)

    return nc


def _augmented(xp32, xg32, w32):
    """U26 [B,N,26] (j side) and V26 [B,N,26] (i side), both w^2-scaled
    with a sqrt(c) pad so psum = (w_i w_j)^2 d2p d2g + c."""
    xp = xp32.astype(np.float64)
    xg = xg32.astype(np.float64)
    w = w32.astype(np.float64)
    sp = (xp * xp).sum(-1)
    sg = (xg * xg).sum(-1)
    ones = np.ones((B, N, 1))
    up = np.concatenate([xp, sp[..., None], ones], -1)
    ug = np.concatenate([xg, sg[..., None], ones], -1)
    vp = np.concatenate([-2.0 * xp, ones, sp[..., None]], -1)
    vg = np.concatenate([-2.0 * xg, ones, sg[..., None]], -1)
    U = np.einsum("bna,bnc->bnac", up, ug).reshape(B, N, 25) * (w ** 2)[..., None]
    V = np.einsum("bna,bnc->bnac", vp, vg).reshape(B, N, 25) * (w ** 2)[..., None]
    U26 = np.concatenate([U, np.full((B, N, 1), SQC)], -1).astype(np.float32)
    V26 = np.concatenate([V, np.full((B, N, 1), SQC)], -1).astype(np.float32)
    return U26, V26


def _host_inputs(U26, V26):
    in_maps = []
    for core in range(8):
        b, h = core // 2, core % 2
        slots = SLOTS_BY_PARITY[h]
        buf = np.zeros((128, UV_COLS), np.float32)
        for s in range(8):
            bi = slots[s]
            u = 16 - bi
            g, vcol = SLOT_GROUP[s]
            r0 = 32 * g
            buf[r0:r0 + 26, vcol:vcol + 128] = V26[b, bi * 128:(bi + 1) * 128].T
            # rhs: quarter-scaled diag block | U tail | optional dummy
            lo = RHS_BASE[s]
            buf[r0:r0 + 26, lo:lo + 128] = 0.25 * U26[b, bi * 128:(bi + 1) * 128].T
            tail = U26[b, (bi + 1) * 128:].T  # [26, (15-bi)*128]
            buf[r0:r0 + 26, lo + 128:lo + u * 128] = tail
            if u % 2 == 1:
                dummy = np.zeros((26, 128), np.float32)
                dummy[25, :] = SQC
                buf[r0:r0 + 26, lo + u * 128:lo + (u + 1) * 128] = dummy
        in_maps.append({"uv": np.ascontiguousarray(buf)})
    return in_maps


def _host_corrections(xp, xg, w):
    """Per-batch within-block corrections, f64.
    Returns (Wfull_dev, Wtrue):
      Wfull_dev[b] = sum over 16 diag blocks of 0.5*sqrt((w_i w_j)^2 d2p d2g + c)
                     over ALL ordered (i, j) incl. i==j  (device content)
      Wtrue[b]     = sum over blocks of w_i w_j dp dg over i != j ordered."""
    X = xp.reshape(B, 16, 128, 3)
    G = xg.reshape(B, 16, 128, 3)
    W = w.reshape(B, 16, 128)
    d2p = ((X[:, :, :, None, :] - X[:, :, None, :, :]) ** 2).sum(-1)
    d2g = ((G[:, :, :, None, :] - G[:, :, None, :, :]) ** 2).sum(-1)
    wp = (W[:, :, :, None] * W[:, :, None, :]) ** 2
    prod = wp * d2p * d2g
    wfull = 0.5 * np.sqrt(prod + C_BIAS).sum(axis=(1, 2, 3))
    m = np.sqrt(prod)
    idx = np.arange(128)
    m[:, :, idx, idx] = 0.0
    wtrue = m.sum(axis=(1, 2, 3))
    return wfull, wtrue


def _host_assemble(xp32, xg32, ht32, w32, P):
    """Alignment loss + analytic bond parts + final scaling (f64)."""
    xp = xp32.astype(np.float64)
    xg = xg32.astype(np.float64)
    ht = ht32.astype(np.float64)
    w = w32.astype(np.float64)

    W = w.sum(axis=1)
    mu = (w[..., None] * xg).sum(axis=1) / W[:, None]
    muGT = (w[..., None] * xp).sum(axis=1) / W[:, None]
    xc = xg - mu[:, None, :]
    xGTc = xp - muGT[:, None, :]
    M = np.einsum("bni,bnj->bij", w[..., None] * xGTc, xc)
    U, _, Vh = np.linalg.svd(M)
    R = U @ Vh
    det = np.linalg.det(R)
    Fm = np.diag([1.0, 1.0, -1.0])
    Rfix = np.einsum("bij,jk,bkl->bil", U, Fm, Vh)
    R = np.where(det[:, None, None] < 0, Rfix, R)
    xalign = np.einsum("bnj,bkj->bnk", xc, R) + muGT[:, None, :]
    lnum = (np.linalg.norm(xp - xalign, axis=-1) * w).sum()
    loss_align = lnum / W.sum()

    sp = (xp * xp).sum(-1)
    sg = (xg * xg).sum(-1)
    wxp = np.einsum("bn,bni->bi", w, xp)
    wxg = np.einsum("bn,bni->bi", w, xg)
    Ap = 2 * (W * (w * sp).sum(1) - (wxp ** 2).sum(1))
    Bg = 2 * (W * (w * sg).sum(1) - (wxg ** 2).sum(1))

    bond = (Ap + Bg - 2 * P).sum() / (W ** 2).sum()
    loss = loss_align + bond
    out = (ht ** 2 + SIGMA_DATA ** 2) / (ht + SIGMA_DATA) ** 2 * loss
    return out.astype(np.float32)


def kernel(xpred_l, xGT_l, ht, w_l):
    global _NC_CACHE
    xp32 = np.ascontiguousarray(np.asarray(xpred_l, dtype=np.float32))
    xg32 = np.ascontiguousarray(np.asarray(xGT_l, dtype=np.float32))
    ht32 = np.asarray(ht, dtype=np.float32)
    w32 = np.ascontiguousarray(np.asarray(w_l, dtype=np.float32))

    if _NC_CACHE is None:
        _NC_CACHE = _build_nc()
    nc = _NC_CACHE

    U26, V26 = _augmented(xp32, xg32, w32)
    in_maps = _host_inputs(U26, V26)
    results = run_bass_kernel_spmd(nc, in_maps, list(range(8))).results

    # Device: res[p, k] = per-partition accumulators (15 per core); every
    # entry already carries its w_i w_j weight, so S_dev = plain sum.
    S_dev = np.zeros(B)
    for core in range(8):
        S_dev[core // 2] += results[core]["res"].astype(np.float64).sum()

    # Dummy columns: 4 per core, each 128x128 entries of exactly sqrt(c).
    dummy_sub = 2 * 4 * 128 * 128 * SQC  # per batch (2 cores)

    xp64 = xp32.astype(np.float64)
    xg64 = xg32.astype(np.float64)
    w64 = w32.astype(np.float64)
    wfull, wtrue = _host_corrections(xp64, xg64, w64)
    P = 2.0 * (S_dev - dummy_sub - wfull) + wtrue

    return _host_assemble(xp32, xg32, ht32, w32, P)


# revision 27
# speedup vs baseline: 1.0157x; 1.0048x over previous
"""Trainium2 Bass kernel for nn_DiffusionLoss (B=4, N=2048).

Decomposition
-------------
loss = align_term + bond_term, pooled over the batch, then scaled by the
per-sample ht factor.

* align term + all O(N) reductions -> host numpy in f64 (tiny).
* bond term: sum_ij w_i w_j (dp_ij - dg_ij)^2 expands to analytic O(N)
  sums plus the cross term P = sum_ij w_i w_j dp_ij dg_ij, which needs the
  full N x N pairwise pass -> device.

Device trick: with augmented 26-vectors
  V_i = [w_i^2 * (vp_i (x) vg_i), sqrt(c)],  U_j = [w_j^2 * (up_j (x) ug_j), sqrt(c)]
one fp32r matmul produces psum_ij = (w_i w_j)^2 d2p_ij d2g_ij + c per
entry, so a single per-entry sqrt gives the fully weighted summand and a
per-partition row-sum accumulator can merge rows from ANY row-blocks.
The +c bias (c = 4) keeps entries positive against rounding noise before
sqrt; its systematic effect (~1e-3 relative) is far inside the 2e-2 gate.

Triangle at 128-row granularity: row-block bi covers columns
[bi*128, 2048).  Its own 128-wide diagonal block is matmul'd against a
quarter-scaled copy of U (sqrt -> half weight), which makes the host
factor a uniform 2x for every device entry; the within-block sums plus
the quarter/bias corrections are reconstructed exactly on the host in
f64 from a cheap O(N*128) pass.  Row-blocks whose column count is an odd
multiple of 128 are padded with one 128-wide dummy column block (zeros
with a sqrt(c) pad row -> each entry contributes exactly sqrt(c), an
exact host-side subtraction), so every matmul piece is 256 or 512 wide:
no fp32r narrow-matmul penalty and all pieces sit inside one PSUM bank.

Pipeline: PE fills one 2048-col half of PSUM (a "generation" = one or
two whole row-block slots) while THREE engines concurrently consume the
previous half, each doing a fused sqrt + row-sum in one instruction:
  ACT  activation(Sqrt, accum_out)            0.833 ns/col
  DVE  tensor_scalar(pow 0.5, op1=add, accum) 1.042 ns/col
  Pool tensor_scalar(pow 0.5, op1=add, accum) 0.833 ns/col (no bubble)
The 2048 cols of each generation are split 512/640/896 (ACT/DVE/Pool) to
equalize finish times.  The ACT Sqrt table is pre-loaded during the DMA
window so no reader pays the 1.4us table-load.

Sharding: one program for all 8 cores; core c -> batch c//2, parity c%2
picks 8 of the 16 row-blocks.  Both parities' padded widths are the same
multiset [16,14,12,10,8,6,4,2] x 128, so a single instruction stream
serves all cores; only the host-built uv payload differs.  Slots are
paired (widest+narrowest) into four 26-row groups stacked on partitions
0/32/64/96 of one [128, 2560] fp32r SBUF tensor, which keeps the DMA
cost at 2560 B/partition/queue across the three DGE queues (SP/ACT/Pool)
and satisfies matmul's equal-base-partition constraint for lhsT and rhs.

Raw Bass (no TileContext): all waits are standalone wait_ge instructions
and each compute instruction carries at most one sem update.  The final
result DMAs carry a sem nobody waits on: the engines halt as soon as the
descriptors are issued and the runtime drains the queues.
"""

from contextlib import ExitStack

import numpy as np

import concourse.bass as bass
from concourse import mybir
from concourse.bass_utils import run_bass_kernel_spmd

B = 4
N = 2048
SIGMA_DATA = 16.0
C_BIAS = 4.0
SQC = 2.0  # sqrt(C_BIAS)

F32 = mybir.dt.float32
F32R = mybir.dt.float32r

# Row-block slots per parity, ascending bi == descending width.
SLOTS_BY_PARITY = {
    0: [0, 3, 4, 7, 8, 11, 12, 15],
    1: [1, 2, 5, 6, 9, 10, 13, 14],
}
# Padded widths (units of 128 cols) per slot index — identical for both
# parities: u = 16-bi rounded up to even.
PW = [16, 14, 12, 10, 8, 6, 4, 2]
# Three 26-partition groups at base partitions 0/32/64 (matmul requires
# lhsT/rhs base partition in {0, 32, 64}).  Each group: its slots' V
# blocks first, then their rhs spans (small slots' rhs early so the DMA
# stream in ascending column order matches PE consumption order).
GROUPS = {0: [0, 5, 7], 1: [1, 3], 2: [2, 4, 6]}
SLOT_GROUP = {}   # slot -> (group, v_col)
RHS_BASE = {}     # slot -> rhs col base within group
UV_COLS = 3456
for _g, _ss in GROUPS.items():
    for _i, _s in enumerate(_ss):
        SLOT_GROUP[_s] = (_g, 128 * _i)
_rhs_order = {0: [0, 7, 5], 1: [1, 3], 2: [4, 2, 6]}
for _g, _ss in _rhs_order.items():
    _off = 128 * len(GROUPS[_g])
    for _s in _ss:
        RHS_BASE[_s] = _off
        _off += PW[_s] * 128

# Generations: (psum_base, [slot indices]).  gen0 is the smallest slot so
# the ACT reader chain starts as early as possible.
GENS = [(0, [4]), (2048, [0]), (0, [1, 7]), (2048, [2, 6]), (0, [3]),
        (2048, [5])]
GEN_COLS = [1024, 2048, 2048, 2048, 1280, 768]
# Real TRN2: ACT Sqrt(negative) is NaN and fp32r noise reaches ~1e2 on
# large-magnitude rows, so every entry is clamped first.  DVE relu-copies
# PSUM into SBUF (the only engine that can both read PSUM and clamp);
# ACT then does the fused sqrt + row-sum from SBUF.
READER_SPLIT = [(c,) for c in GEN_COLS]

# DMA stripes: (queue, col_lo, col_hi) in issue order per queue.
STRIPES = {
    "sp": [(0, 128), (384, 640), (1152, 1536), (2048, 2432), (2944, 3456)],
    "act": [(128, 384), (1536, 2048)],
    "pool": [(640, 1152), (2432, 2944)],
}


def _piece_plan():
    """Matmul pieces: (slot, sbuf_off_in_rhs, psum_abs_off, width), in PE
    issue order.  gen0's slot is split 256/256/512/512/512 for an earlier
    start; everything else splits at PSUM bank boundaries into 256/512s."""
    pieces = []
    for gi, (base, slots) in enumerate(GENS):
        off_in_gen = 0
        for s in slots:
            cols = PW[s] * 128
            pos = 0
            while pos < cols:
                abs_off = base + off_in_gen + pos
                room = 512 - (abs_off % 512)
                w = min(512, cols - pos, room)
                if gi == 0:
                    w = min(w, 256)  # 256-wide pieces so PE starts early
                pieces.append((s, pos, abs_off, w))
                pos += w
            off_in_gen += cols
    return pieces


PIECES = _piece_plan()
# cumulative piece count at end of each generation
M_CUM = []
_n = 0
for _base, _slots in GENS:
    for _s in _slots:
        _cols = PW[_s] * 128
        _pos = 0
        while _pos < _cols:
            _pc_abs = PIECES[_n][2]
            _n += 1
            _pos += PIECES[_n - 1][3]
    M_CUM.append(_n)
assert M_CUM[-1] == len(PIECES), M_CUM


def _stripe_for(col):
    """(queue, index>=1) of the stripe containing col."""
    for q, lst in STRIPES.items():
        for i, (lo, hi) in enumerate(lst):
            if lo <= col < hi:
                return q, i + 1
    raise AssertionError(col)


def _piece_stripe_targets():
    """Per piece, the set of (queue, stripe_index) stripes it needs.  DMAs
    on one queue can complete out of order, so each stripe has its own
    semaphore and pieces wait on exactly the stripes they touch."""
    targets = []
    for s, pos, abs_off, w in PIECES:
        need = set()
        g, vcol = SLOT_GROUP[s]
        need.add(_stripe_for(vcol))
        lo = RHS_BASE[s] + pos
        col = lo
        while col < lo + w:
            q, i = _stripe_for(col)
            need.add((q, i))
            col = STRIPES[q][i - 1][1]
        targets.append(sorted(need))
    return targets


PIECE_TARGETS = _piece_stripe_targets()


def _reader_units():
    """Flat list of reader units (psum_lo, psum_hi, pe_target, clamp_off,
    gen).  gen0 is split in two so the first clamp starts after only two
    matmul pieces, pulling the whole saturated DVE chain earlier."""
    units = []
    off = 0
    for gi, (base, slots) in enumerate(GENS):
        lo_pc = M_CUM[gi - 1] if gi else 0
        gen_pieces = [(k + 1, PIECES[k][2], PIECES[k][2] + PIECES[k][3])
                      for k in range(lo_pc, M_CUM[gi])]
        if gi == 0:
            ranges = [(base, base + 256), (base + 256, base + 512),
                      (base + 512, base + GEN_COLS[0])]
        else:
            ranges = [(base, base + GEN_COLS[gi])]
        for lo, hi in ranges:
            need = max(idx for idx, plo, phi in gen_pieces if plo < hi and phi > lo)
            units.append((lo, hi, need, off, gi))
            off += hi - lo
    return units


READER_UNITS = _reader_units()
# number of DVE units completed at the end of each generation (for PE's
# psum-half-reuse waits)
DVE_CUM = []
for _gi in range(len(GENS)):
    DVE_CUM.append(sum(1 for u in READER_UNITS if u[4] <= _gi))

_NC_CACHE = None


def _build_nc():
    nc = bass.Bass("TRN2", target_bir_lowering=False, debug=False, num_devices=8)

    uv = nc.declare_dram_parameter("uv", [128, UV_COLS], F32R, isOutput=False)
    res = nc.declare_dram_parameter("res", [128, 8], F32, isOutput=True)

    with (
        nc.sbuf_tensor([128, UV_COLS], F32R) as uv_t,
        nc.sbuf_tensor([128, 9216], F32) as scl,
        nc.sbuf_tensor([128, 16], F32) as res_t,
        nc.sbuf_tensor([128, 1], F32) as bias_t,
        nc.psum_tensor([128, 4096], F32) as ps,
        ExitStack() as stack,
        nc.Block() as block,
    ):
        names = ["pe_s", "act_s", "dve_s", "bias_s", "dout"]
        for q, lst in STRIPES.items():
            names += [f"q_{q}_{i + 1}" for i in range(len(lst))]
        sems = {name: stack.enter_context(nc.semaphore(name)) for name in names}
        pe_s, act_s, dve_s, bias_s, dout = (sems[k] for k in
                                            ("pe_s", "act_s", "dve_s",
                                             "bias_s", "dout"))
        qsem = {(q, i + 1): sems[f"q_{q}_{i + 1}"]
                for q, lst in STRIPES.items() for i in range(len(lst))}

        def rhs_ap(s, pos, w):
            g, _ = SLOT_GROUP[s]
            lo = RHS_BASE[s] + pos
            return uv_t[32 * g: 32 * g + 26, lo: lo + w]

        def lhs_ap(s):
            g, vcol = SLOT_GROUP[s]
            return uv_t[32 * g: 32 * g + 26, vcol: vcol + 128]

        @block.sync
        def _(sync):
            for i, (lo, hi) in enumerate(STRIPES["sp"]):
                sync.dma_start(out=uv_t[:, lo:hi], in_=uv[:, lo:hi]).then_inc(
                    qsem[("sp", i + 1)], 16)

        @block.tensor
        def _(tensor):
            waited = set()
            for gi in range(len(GENS)):
                lo_pc = M_CUM[gi - 1] if gi else 0
                for k in range(lo_pc, M_CUM[gi]):
                    s, pos, abs_off, w = PIECES[k]
                    if k == lo_pc and gi >= 2:
                        # psum half reuse: the clamp is the psum reader
                        tensor.wait_ge(dve_s, DVE_CUM[gi - 2])
                    for st in PIECE_TARGETS[k]:
                        if st not in waited:
                            tensor.wait_ge(qsem[st], 16)
                            waited.add(st)
                    nc.tensor.matmul(
                        ps[:, abs_off: abs_off + w],
                        lhs_ap(s),
                        rhs_ap(s, pos, w),
                        start=True,
                        stop=True,
                    ).then_inc(pe_s, 1)

        @block.gpsimd
        def _(gp):
            gp.memset(bias_t[:, :], 0.0).then_inc(bias_s, 1)
            for i, (lo, hi) in enumerate(STRIPES["pool"]):
                gp.dma_start(out=uv_t[:, lo:hi], in_=uv[:, lo:hi]).then_inc(
                    qsem[("pool", i + 1)], 16)

        @block.vector
        def _(vector):
            for lo, hi, tgt, coff, gi in READER_UNITS:
                vector.wait_ge(pe_s, tgt)
                nc.vector.tensor_scalar_max(
                    scl[:, coff: coff + (hi - lo)],
                    ps[:, lo:hi],
                    0.0,
                ).then_inc(dve_s, 1)

        @block.scalar
        def _(scalar):
            for i, (lo, hi) in enumerate(STRIPES["act"]):
                scalar.dma_start(out=uv_t[:, lo:hi], in_=uv[:, lo:hi]).then_inc(
                    qsem[("act", i + 1)], 16)
            # pre-load the Sqrt activation table during the DMA window
            scalar.wait_ge(bias_s, 1)
            nc.scalar.activation(
                out=res_t[:, 15:16], in_=bias_t[:, 0:1],
                func=mybir.ActivationFunctionType.Sqrt,
                bias=bias_t[:, 0:1],
            )
            for ui, (lo, hi, tgt, coff, gi) in enumerate(READER_UNITS):
                scalar.wait_ge(dve_s, ui + 1)
                nc.scalar.activation(
                    out=scl[:, coff: coff + (hi - lo)],
                    in_=scl[:, coff: coff + (hi - lo)],
                    func=mybir.ActivationFunctionType.Sqrt,
                    bias=bias_t[:, 0:1],
                    accum_out=res_t[:, ui: ui + 1],
                ).then_inc(act_s, 1)
            scalar.wait_ge(act_s, len(READER_UNITS))
            scalar.dma_start(out=res[:, 0:len(READER_UNITS)],
                             in_=res_t[:, 0:len(READER_UNITS)]).then_inc(dout, 16)

    return nc


def _augmented(xp32, xg32, w32):
    """U26 [B,N,26] (j side) and V26 [B,N,26] (i side), both w^2-scaled
    with a sqrt(c) pad so psum = (w_i w_j)^2 d2p d2g + c."""
    xp = xp32.astype(np.float64)
    xg = xg32.astype(np.float64)
    w = w32.astype(np.float64)
    sp = (xp * xp).sum(-1)
    sg = (xg * xg).sum(-1)
    ones = np.ones((B, N, 1))
    up = np.concatenate([xp, sp[..., None], ones], -1)
    ug = np.concatenate([xg, sg[..., None], ones], -1)
    vp = np.concatenate([-2.0 * xp, ones, sp[..., None]], -1)
    vg = np.concatenate([-2.0 * xg, ones, sg[..., None]], -1)
    U = np.einsum("bna,bnc->bnac", up, ug).reshape(B, N, 25) * (w ** 2)[..., None]
    V = np.einsum("bna,bnc->bnac", vp, vg).reshape(B, N, 25) * (w ** 2)[..., None]
    U26 = np.concatenate([U, np.full((B, N, 1), SQC)], -1).astype(np.float32)
    V26 = np.concatenate([V, np.full((B, N, 1), SQC)], -1).astype(np.float32)
    return U26, V26


def _host_inputs(U26, V26):
    in_maps = []
    for core in range(8):
        b, h = core // 2, core % 2
        slots = SLOTS_BY_PARITY[h]
        buf = np.zeros((128, UV_COLS), np.float32)
        for s in range(8):
            bi = slots[s]
            u = 16 - bi
            g, vcol = SLOT_GROUP[s]
            r0 = 32 * g
            buf[r0:r0 + 26, vcol:vcol + 128] = V26[b, bi * 128:(bi + 1) * 128].T
            # rhs: quarter-scaled diag block | U tail | optional dummy
            lo = RHS_BASE[s]
            buf[r0:r0 + 26, lo:lo + 128] = 0.25 * U26[b, bi * 128:(bi + 1) * 128].T
            tail = U26[b, (bi + 1) * 128:].T  # [26, (15-bi)*128]
            buf[r0:r0 + 26, lo + 128:lo + u * 128] = tail
            if u % 2 == 1:
                dummy = np.zeros((26, 128), np.float32)
                dummy[25, :] = SQC
                buf[r0:r0 + 26, lo + u * 128:lo + (u + 1) * 128] = dummy
        in_maps.append({"uv": np.ascontiguousarray(buf)})
    return in_maps


def _host_corrections(xp, xg, w):
    """Per-batch within-block corrections, f64.
    Returns (Wfull_dev, Wtrue):
      Wfull_dev[b] = sum over 16 diag blocks of 0.5*sqrt((w_i w_j)^2 d2p d2g + c)
                     over ALL ordered (i, j) incl. i==j  (device content)
      Wtrue[b]     = sum over blocks of w_i w_j dp dg over i != j ordered."""
    X = xp.reshape(B, 16, 128, 3)
    G = xg.reshape(B, 16, 128, 3)
    W = w.reshape(B, 16, 128)
    d2p = ((X[:, :, :, None, :] - X[:, :, None, :, :]) ** 2).sum(-1)
    d2g = ((G[:, :, :, None, :] - G[:, :, None, :, :]) ** 2).sum(-1)
    wp = (W[:, :, :, None] * W[:, :, None, :]) ** 2
    prod = wp * d2p * d2g
    wfull = 0.5 * np.sqrt(prod + C_BIAS).sum(axis=(1, 2, 3))
    m = np.sqrt(prod)
    idx = np.arange(128)
    m[:, :, idx, idx] = 0.0
    wtrue = m.sum(axis=(1, 2, 3))
    return wfull, wtrue


def _host_assemble(xp32, xg32, ht32, w32, P):
    """Alignment loss + analytic bond parts + final scaling (f64)."""
    xp = xp32.astype(np.float64)
    xg = xg32.astype(np.float64)
    ht = ht32.astype(np.float64)
    w = w32.astype(np.float64)

    W = w.sum(axis=1)
    mu = (w[..., None] * xg).sum(axis=1) / W[:, None]
    muGT = (w[..., None] * xp).sum(axis=1) / W[:, None]
    xc = xg - mu[:, None, :]
    xGTc = xp - muGT[:, None, :]
    M = np.einsum("bni,bnj->bij", w[..., None] * xGTc, xc)
    U, _, Vh = np.linalg.svd(M)
    R = U @ Vh
    det = np.linalg.det(R)
    Fm = np.diag([1.0, 1.0, -1.0])
    Rfix = np.einsum("bij,jk,bkl->bil", U, Fm, Vh)
    R = np.where(det[:, None, None] < 0, Rfix, R)
    xalign = np.einsum("bnj,bkj->bnk", xc, R) + muGT[:, None, :]
    lnum = (np.linalg.norm(xp - xalign, axis=-1) * w).sum()
    loss_align = lnum / W.sum()

    sp = (xp * xp).sum(-1)
    sg = (xg * xg).sum(-1)
    wxp = np.einsum("bn,bni->bi", w, xp)
    wxg = np.einsum("bn,bni->bi", w, xg)
    Ap = 2 * (W * (w * sp).sum(1) - (wxp ** 2).sum(1))
    Bg = 2 * (W * (w * sg).sum(1) - (wxg ** 2).sum(1))

    bond = (Ap + Bg - 2 * P).sum() / (W ** 2).sum()
    loss = loss_align + bond
    out = (ht ** 2 + SIGMA_DATA ** 2) / (ht + SIGMA_DATA) ** 2 * loss
    return out.astype(np.float32)


def kernel(xpred_l, xGT_l, ht, w_l):
    global _NC_CACHE
    xp32 = np.ascontiguousarray(np.asarray(xpred_l, dtype=np.float32))
    xg32 = np.ascontiguousarray(np.asarray(xGT_l, dtype=np.float32))
    ht32 = np.asarray(ht, dtype=np.float32)
    w32 = np.ascontiguousarray(np.asarray(w_l, dtype=np.float32))

    if _NC_CACHE is None:
        _NC_CACHE = _build_nc()
    nc = _NC_CACHE

    U26, V26 = _augmented(xp32, xg32, w32)
    in_maps = _host_inputs(U26, V26)
    results = run_bass_kernel_spmd(nc, in_maps, list(range(8))).results

    # Device: res[p, k] = per-partition accumulators (15 per core); every
    # entry already carries its w_i w_j weight, so S_dev = plain sum.
    S_dev = np.zeros(B)
    for core in range(8):
        S_dev[core // 2] += results[core]["res"].astype(np.float64).sum()

    # Dummy columns: 4 per core, each 128x128 entries of exactly sqrt(c).
    dummy_sub = 2 * 4 * 128 * 128 * SQC  # per batch (2 cores)

    xp64 = xp32.astype(np.float64)
    xg64 = xg32.astype(np.float64)
    w64 = w32.astype(np.float64)
    wfull, wtrue = _host_corrections(xp64, xg64, w64)
    P = 2.0 * (S_dev - dummy_sub - wfull) + wtrue

    return _host_assemble(xp32, xg32, ht32, w32, P)


# revision 28
# speedup vs baseline: 1.0267x; 1.0108x over previous
"""Trainium2 Bass kernel for nn_DiffusionLoss (B=4, N=2048).

Decomposition
-------------
loss = align_term + bond_term, pooled over the batch, then scaled by the
per-sample ht factor.

* align term + all O(N) reductions -> host numpy in f64 (tiny).
* bond term: sum_ij w_i w_j (dp_ij - dg_ij)^2 expands to analytic O(N)
  sums plus the cross term P = sum_ij w_i w_j dp_ij dg_ij, which needs the
  full N x N pairwise pass -> device.

Device trick: with augmented 26-vectors
  V_i = [w_i^2 * (vp_i (x) vg_i), sqrt(c)],  U_j = [w_j^2 * (up_j (x) ug_j), sqrt(c)]
one fp32r matmul produces psum_ij = (w_i w_j)^2 d2p_ij d2g_ij + c per
entry, so a single per-entry sqrt gives the fully weighted summand and a
per-partition row-sum accumulator can merge rows from ANY row-blocks.
The +c bias (c = 4) keeps entries positive against rounding noise before
sqrt; its systematic effect (~1e-3 relative) is far inside the 2e-2 gate.

Triangle at 128-row granularity: row-block bi covers columns
[bi*128, 2048).  Its own 128-wide diagonal block is matmul'd against a
quarter-scaled copy of U (sqrt -> half weight), which makes the host
factor a uniform 2x for every device entry; the within-block sums plus
the quarter/bias corrections are reconstructed exactly on the host in
f64 from a cheap O(N*128) pass.  Row-blocks whose column count is an odd
multiple of 128 are padded with one 128-wide dummy column block (zeros
with a sqrt(c) pad row -> each entry contributes exactly sqrt(c), an
exact host-side subtraction), so every matmul piece is 256 or 512 wide:
no fp32r narrow-matmul penalty and all pieces sit inside one PSUM bank.

Pipeline: PE fills one 2048-col half of PSUM (a "generation" = one or
two whole row-block slots) while THREE engines concurrently consume the
previous half, each doing a fused sqrt + row-sum in one instruction:
  ACT  activation(Sqrt, accum_out)            0.833 ns/col
  DVE  tensor_scalar(pow 0.5, op1=add, accum) 1.042 ns/col
  Pool tensor_scalar(pow 0.5, op1=add, accum) 0.833 ns/col (no bubble)
The 2048 cols of each generation are split 512/640/896 (ACT/DVE/Pool) to
equalize finish times.  The ACT Sqrt table is pre-loaded during the DMA
window so no reader pays the 1.4us table-load.

Sharding: one program for all 8 cores; core c -> batch c//2, parity c%2
picks 8 of the 16 row-blocks.  Both parities' padded widths are the same
multiset [16,14,12,10,8,6,4,2] x 128, so a single instruction stream
serves all cores; only the host-built uv payload differs.  Slots are
paired (widest+narrowest) into four 26-row groups stacked on partitions
0/32/64/96 of one [128, 2560] fp32r SBUF tensor, which keeps the DMA
cost at 2560 B/partition/queue across the three DGE queues (SP/ACT/Pool)
and satisfies matmul's equal-base-partition constraint for lhsT and rhs.

Raw Bass (no TileContext): all waits are standalone wait_ge instructions
and each compute instruction carries at most one sem update.  The final
result DMAs carry a sem nobody waits on: the engines halt as soon as the
descriptors are issued and the runtime drains the queues.
"""

from contextlib import ExitStack

import numpy as np

import concourse.bass as bass
from concourse import mybir
from concourse.bass_utils import run_bass_kernel_spmd

B = 4
N = 2048
SIGMA_DATA = 16.0
C_BIAS = 4.0
SQC = 2.0  # sqrt(C_BIAS)

F32 = mybir.dt.float32
F32R = mybir.dt.float32r

# Row-block slots per parity, ascending bi == descending width.
SLOTS_BY_PARITY = {
    0: [0, 3, 4, 7, 8, 11, 12, 15],
    1: [1, 2, 5, 6, 9, 10, 13, 14],
}
# Padded widths (units of 128 cols) per slot index — identical for both
# parities: u = 16-bi rounded up to even.
PW = [16, 14, 12, 10, 8, 6, 4, 2]
# Three 26-partition groups at base partitions 0/32/64 (matmul requires
# lhsT/rhs base partition in {0, 32, 64}).  Each group: its slots' V
# blocks first, then their rhs spans (small slots' rhs early so the DMA
# stream in ascending column order matches PE consumption order).
GROUPS = {0: [0, 5, 7], 1: [1, 3], 2: [2, 4, 6]}
SLOT_GROUP = {}   # slot -> (group, v_col)
RHS_BASE = {}     # slot -> rhs col base within group
UV_COLS = 3456
for _g, _ss in GROUPS.items():
    for _i, _s in enumerate(_ss):
        SLOT_GROUP[_s] = (_g, 128 * _i)
_rhs_order = {0: [0, 7, 5], 1: [1, 3], 2: [4, 2, 6]}
for _g, _ss in _rhs_order.items():
    _off = 128 * len(GROUPS[_g])
    for _s in _ss:
        RHS_BASE[_s] = _off
        _off += PW[_s] * 128

# Generations: (psum_base, [slot indices]).  gen0 is the smallest slot so
# the ACT reader chain starts as early as possible.
GENS = [(0, [4]), (2048, [0]), (0, [1, 7]), (2048, [2, 6]), (0, [3, 5])]
GEN_COLS = [1024, 2048, 2048, 2048, 2048]
# Real TRN2: ACT Sqrt(negative) is NaN and fp32r noise reaches ~1e2 on
# large-magnitude rows, so every entry is clamped first.  DVE relu-copies
# PSUM into SBUF (the only engine that can both read PSUM and clamp);
# ACT then does the fused sqrt + row-sum from SBUF.
READER_SPLIT = [(c,) for c in GEN_COLS]

# DMA stripes: (queue, col_lo, col_hi) in issue order per queue.
STRIPES = {
    "sp": [(0, 128), (384, 640), (1152, 1536), (2048, 2432), (2944, 3456)],
    "act": [(128, 384), (1536, 2048)],
    "pool": [(640, 1152), (2432, 2944)],
}


def _piece_plan():
    """Matmul pieces: (slot, sbuf_off_in_rhs, psum_abs_off, width), in PE
    issue order.  gen0's slot is split 256/256/512/512/512 for an earlier
    start; everything else splits at PSUM bank boundaries into 256/512s."""
    pieces = []
    for gi, (base, slots) in enumerate(GENS):
        off_in_gen = 0
        for s in slots:
            cols = PW[s] * 128
            pos = 0
            while pos < cols:
                abs_off = base + off_in_gen + pos
                room = 512 - (abs_off % 512)
                w = min(512, cols - pos, room)
                if gi == 0:
                    w = min(w, 256)  # 256-wide pieces so PE starts early
                pieces.append((s, pos, abs_off, w))
                pos += w
            off_in_gen += cols
    return pieces


PIECES = _piece_plan()
# cumulative piece count at end of each generation
M_CUM = []
_n = 0
for _base, _slots in GENS:
    for _s in _slots:
        _cols = PW[_s] * 128
        _pos = 0
        while _pos < _cols:
            _pc_abs = PIECES[_n][2]
            _n += 1
            _pos += PIECES[_n - 1][3]
    M_CUM.append(_n)
assert M_CUM[-1] == len(PIECES), M_CUM


def _stripe_for(col):
    """(queue, index>=1) of the stripe containing col."""
    for q, lst in STRIPES.items():
        for i, (lo, hi) in enumerate(lst):
            if lo <= col < hi:
                return q, i + 1
    raise AssertionError(col)


def _piece_stripe_targets():
    """Per piece, the set of (queue, stripe_index) stripes it needs.  DMAs
    on one queue can complete out of order, so each stripe has its own
    semaphore and pieces wait on exactly the stripes they touch."""
    targets = []
    for s, pos, abs_off, w in PIECES:
        need = set()
        g, vcol = SLOT_GROUP[s]
        need.add(_stripe_for(vcol))
        lo = RHS_BASE[s] + pos
        col = lo
        while col < lo + w:
            q, i = _stripe_for(col)
            need.add((q, i))
            col = STRIPES[q][i - 1][1]
        targets.append(sorted(need))
    return targets


PIECE_TARGETS = _piece_stripe_targets()


def _reader_units():
    """Flat list of reader units (psum_lo, psum_hi, pe_target, clamp_off,
    gen).  gen0 is split in two so the first clamp starts after only two
    matmul pieces, pulling the whole saturated DVE chain earlier."""
    units = []
    off = 0
    for gi, (base, slots) in enumerate(GENS):
        lo_pc = M_CUM[gi - 1] if gi else 0
        gen_pieces = [(k + 1, PIECES[k][2], PIECES[k][2] + PIECES[k][3])
                      for k in range(lo_pc, M_CUM[gi])]
        if gi == 0:
            ranges = [(base, base + 256), (base + 256, base + 512),
                      (base + 512, base + GEN_COLS[0])]
        else:
            ranges = [(base, base + GEN_COLS[gi])]
        for lo, hi in ranges:
            need = max(idx for idx, plo, phi in gen_pieces if plo < hi and phi > lo)
            units.append((lo, hi, need, off, gi))
            off += hi - lo
    return units


READER_UNITS = _reader_units()
# number of DVE units completed at the end of each generation (for PE's
# psum-half-reuse waits)
DVE_CUM = []
for _gi in range(len(GENS)):
    DVE_CUM.append(sum(1 for u in READER_UNITS if u[4] <= _gi))

_NC_CACHE = None


def _build_nc():
    nc = bass.Bass("TRN2", target_bir_lowering=False, debug=False, num_devices=8)

    uv = nc.declare_dram_parameter("uv", [128, UV_COLS], F32R, isOutput=False)
    res = nc.declare_dram_parameter("res", [128, 7], F32, isOutput=True)

    with (
        nc.sbuf_tensor([128, UV_COLS], F32R) as uv_t,
        nc.sbuf_tensor([128, 9216], F32) as scl,
        nc.sbuf_tensor([128, 16], F32) as res_t,
        nc.sbuf_tensor([128, 1], F32) as bias_t,
        nc.psum_tensor([128, 4096], F32) as ps,
        ExitStack() as stack,
        nc.Block() as block,
    ):
        names = ["pe_s", "act_s", "dve_s", "bias_s", "dout"]
        for q, lst in STRIPES.items():
            names += [f"q_{q}_{i + 1}" for i in range(len(lst))]
        sems = {name: stack.enter_context(nc.semaphore(name)) for name in names}
        pe_s, act_s, dve_s, bias_s, dout = (sems[k] for k in
                                            ("pe_s", "act_s", "dve_s",
                                             "bias_s", "dout"))
        qsem = {(q, i + 1): sems[f"q_{q}_{i + 1}"]
                for q, lst in STRIPES.items() for i in range(len(lst))}

        def rhs_ap(s, pos, w):
            g, _ = SLOT_GROUP[s]
            lo = RHS_BASE[s] + pos
            return uv_t[32 * g: 32 * g + 26, lo: lo + w]

        def lhs_ap(s):
            g, vcol = SLOT_GROUP[s]
            return uv_t[32 * g: 32 * g + 26, vcol: vcol + 128]

        @block.sync
        def _(sync):
            for i, (lo, hi) in enumerate(STRIPES["sp"]):
                sync.dma_start(out=uv_t[:, lo:hi], in_=uv[:, lo:hi]).then_inc(
                    qsem[("sp", i + 1)], 16)

        @block.tensor
        def _(tensor):
            waited = set()
            for gi in range(len(GENS)):
                lo_pc = M_CUM[gi - 1] if gi else 0
                for k in range(lo_pc, M_CUM[gi]):
                    s, pos, abs_off, w = PIECES[k]
                    if k == lo_pc and gi >= 2:
                        # psum half reuse: the clamp is the psum reader
                        tensor.wait_ge(dve_s, DVE_CUM[gi - 2])
                    for st in PIECE_TARGETS[k]:
                        if st not in waited:
                            tensor.wait_ge(qsem[st], 16)
                            waited.add(st)
                    nc.tensor.matmul(
                        ps[:, abs_off: abs_off + w],
                        lhs_ap(s),
                        rhs_ap(s, pos, w),
                        start=True,
                        stop=True,
                    ).then_inc(pe_s, 1)

        @block.gpsimd
        def _(gp):
            gp.memset(bias_t[:, :], 0.0).then_inc(bias_s, 1)
            for i, (lo, hi) in enumerate(STRIPES["pool"]):
                gp.dma_start(out=uv_t[:, lo:hi], in_=uv[:, lo:hi]).then_inc(
                    qsem[("pool", i + 1)], 16)

        @block.vector
        def _(vector):
            for lo, hi, tgt, coff, gi in READER_UNITS:
                vector.wait_ge(pe_s, tgt)
                nc.vector.tensor_scalar_max(
                    scl[:, coff: coff + (hi - lo)],
                    ps[:, lo:hi],
                    0.0,
                ).then_inc(dve_s, 1)

        @block.scalar
        def _(scalar):
            for i, (lo, hi) in enumerate(STRIPES["act"]):
                scalar.dma_start(out=uv_t[:, lo:hi], in_=uv[:, lo:hi]).then_inc(
                    qsem[("act", i + 1)], 16)
            # pre-load the Sqrt activation table during the DMA window
            scalar.wait_ge(bias_s, 1)
            nc.scalar.activation(
                out=res_t[:, 15:16], in_=bias_t[:, 0:1],
                func=mybir.ActivationFunctionType.Sqrt,
                bias=bias_t[:, 0:1],
            )
            for ui, (lo, hi, tgt, coff, gi) in enumerate(READER_UNITS):
                scalar.wait_ge(dve_s, ui + 1)
                nc.scalar.activation(
                    out=scl[:, coff: coff + (hi - lo)],
                    in_=scl[:, coff: coff + (hi - lo)],
                    func=mybir.ActivationFunctionType.Sqrt,
                    bias=bias_t[:, 0:1],
                    accum_out=res_t[:, ui: ui + 1],
                ).then_inc(act_s, 1)
            scalar.wait_ge(act_s, len(READER_UNITS))
            scalar.dma_start(out=res[:, 0:len(READER_UNITS)],
                             in_=res_t[:, 0:len(READER_UNITS)]).then_inc(dout, 16)

    return nc


def _augmented(xp32, xg32, w32):
    """U26 [B,N,26] (j side) and V26 [B,N,26] (i side), both w^2-scaled
    with a sqrt(c) pad so psum = (w_i w_j)^2 d2p d2g + c."""
    xp = xp32.astype(np.float64)
    xg = xg32.astype(np.float64)
    w = w32.astype(np.float64)
    sp = (xp * xp).sum(-1)
    sg = (xg * xg).sum(-1)
    ones = np.ones((B, N, 1))
    up = np.concatenate([xp, sp[..., None], ones], -1)
    ug = np.concatenate([xg, sg[..., None], ones], -1)
    vp = np.concatenate([-2.0 * xp, ones, sp[..., None]], -1)
    vg = np.concatenate([-2.0 * xg, ones, sg[..., None]], -1)
    U = np.einsum("bna,bnc->bnac", up, ug).reshape(B, N, 25) * (w ** 2)[..., None]
    V = np.einsum("bna,bnc->bnac", vp, vg).reshape(B, N, 25) * (w ** 2)[..., None]
    U26 = np.concatenate([U, np.full((B, N, 1), SQC)], -1).astype(np.float32)
    V26 = np.concatenate([V, np.full((B, N, 1), SQC)], -1).astype(np.float32)
    return U26, V26


def _host_inputs(U26, V26):
    in_maps = []
    for core in range(8):
        b, h = core // 2, core % 2
        slots = SLOTS_BY_PARITY[h]
        buf = np.zeros((128, UV_COLS), np.float32)
        for s in range(8):
            bi = slots[s]
            u = 16 - bi
            g, vcol = SLOT_GROUP[s]
            r0 = 32 * g
            buf[r0:r0 + 26, vcol:vcol + 128] = V26[b, bi * 128:(bi + 1) * 128].T
            # rhs: quarter-scaled diag block | U tail | optional dummy
            lo = RHS_BASE[s]
            buf[r0:r0 + 26, lo:lo + 128] = 0.25 * U26[b, bi * 128:(bi + 1) * 128].T
            tail = U26[b, (bi + 1) * 128:].T  # [26, (15-bi)*128]
            buf[r0:r0 + 26, lo + 128:lo + u * 128] = tail
            if u % 2 == 1:
                dummy = np.zeros((26, 128), np.float32)
                dummy[25, :] = SQC
                buf[r0:r0 + 26, lo + u * 128:lo + (u + 1) * 128] = dummy
        in_maps.append({"uv": np.ascontiguousarray(buf)})
    return in_maps


def _host_corrections(xp, xg, w):
    """Per-batch within-block corrections, f64.
    Returns (Wfull_dev, Wtrue):
      Wfull_dev[b] = sum over 16 diag blocks of 0.5*sqrt((w_i w_j)^2 d2p d2g + c)
                     over ALL ordered (i, j) incl. i==j  (device content)
      Wtrue[b]     = sum over blocks of w_i w_j dp dg over i != j ordered."""
    X = xp.reshape(B, 16, 128, 3)
    G = xg.reshape(B, 16, 128, 3)
    W = w.reshape(B, 16, 128)
    d2p = ((X[:, :, :, None, :] - X[:, :, None, :, :]) ** 2).sum(-1)
    d2g = ((G[:, :, :, None, :] - G[:, :, None, :, :]) ** 2).sum(-1)
    wp = (W[:, :, :, None] * W[:, :, None, :]) ** 2
    prod = wp * d2p * d2g
    wfull = 0.5 * np.sqrt(prod + C_BIAS).sum(axis=(1, 2, 3))
    m = np.sqrt(prod)
    idx = np.arange(128)
    m[:, :, idx, idx] = 0.0
    wtrue = m.sum(axis=(1, 2, 3))
    return wfull, wtrue


def _host_assemble(xp32, xg32, ht32, w32, P):
    """Alignment loss + analytic bond parts + final scaling (f64)."""
    xp = xp32.astype(np.float64)
    xg = xg32.astype(np.float64)
    ht = ht32.astype(np.float64)
    w = w32.astype(np.float64)

    W = w.sum(axis=1)
    mu = (w[..., None] * xg).sum(axis=1) / W[:, None]
    muGT = (w[..., None] * xp).sum(axis=1) / W[:, None]
    xc = xg - mu[:, None, :]
    xGTc = xp - muGT[:, None, :]
    M = np.einsum("bni,bnj->bij", w[..., None] * xGTc, xc)
    U, _, Vh = np.linalg.svd(M)
    R = U @ Vh
    det = np.linalg.det(R)
    Fm = np.diag([1.0, 1.0, -1.0])
    Rfix = np.einsum("bij,jk,bkl->bil", U, Fm, Vh)
    R = np.where(det[:, None, None] < 0, Rfix, R)
    xalign = np.einsum("bnj,bkj->bnk", xc, R) + muGT[:, None, :]
    lnum = (np.linalg.norm(xp - xalign, axis=-1) * w).sum()
    loss_align = lnum / W.sum()

    sp = (xp * xp).sum(-1)
    sg = (xg * xg).sum(-1)
    wxp = np.einsum("bn,bni->bi", w, xp)
    wxg = np.einsum("bn,bni->bi", w, xg)
    Ap = 2 * (W * (w * sp).sum(1) - (wxp ** 2).sum(1))
    Bg = 2 * (W * (w * sg).sum(1) - (wxg ** 2).sum(1))

    bond = (Ap + Bg - 2 * P).sum() / (W ** 2).sum()
    loss = loss_align + bond
    out = (ht ** 2 + SIGMA_DATA ** 2) / (ht + SIGMA_DATA) ** 2 * loss
    return out.astype(np.float32)


def kernel(xpred_l, xGT_l, ht, w_l):
    global _NC_CACHE
    xp32 = np.ascontiguousarray(np.asarray(xpred_l, dtype=np.float32))
    xg32 = np.ascontiguousarray(np.asarray(xGT_l, dtype=np.float32))
    ht32 = np.asarray(ht, dtype=np.float32)
    w32 = np.ascontiguousarray(np.asarray(w_l, dtype=np.float32))

    if _NC_CACHE is None:
        _NC_CACHE = _build_nc()
    nc = _NC_CACHE

    U26, V26 = _augmented(xp32, xg32, w32)
    in_maps = _host_inputs(U26, V26)
    results = run_bass_kernel_spmd(nc, in_maps, list(range(8))).results

    # Device: res[p, k] = per-partition accumulators (15 per core); every
    # entry already carries its w_i w_j weight, so S_dev = plain sum.
    S_dev = np.zeros(B)
    for core in range(8):
        S_dev[core // 2] += results[core]["res"].astype(np.float64).sum()

    # Dummy columns: 4 per core, each 128x128 entries of exactly sqrt(c).
    dummy_sub = 2 * 4 * 128 * 128 * SQC  # per batch (2 cores)

    xp64 = xp32.astype(np.float64)
    xg64 = xg32.astype(np.float64)
    w64 = w32.astype(np.float64)
    wfull, wtrue = _host_corrections(xp64, xg64, w64)
    P = 2.0 * (S_dev - dummy_sub - wfull) + wtrue

    return _host_assemble(xp32, xg32, ht32, w32, P)
